# revision 6
# baseline (speedup 1.0000x reference)
"""Trainium2 Bass kernel for the RNN decoder.

Math (reference):
    tokens = [SOS, target[:,1:]]                       (B, T)
    x      = emb[tokens]                               (B, T, E)
    h_t    = tanh(x_t @ W_ih^T + b_ih + h_{t-1} @ W_hh^T + b_hh)
    out_t  = h_t @ W_out^T + b_out                     (B, V)

Strategy (8 cores, no collectives):
  - Vocab-parallel: W_out/b_out are padded V 32000->32768 and sharded 4096
    rows per core.  Every core redundantly runs the (cheap, sequential)
    recurrence and computes its own vocab slice of the (dominant) output
    projection.
  - Host does the embedding gather + layout transposes; all matmul FLOPs run
    on device in bf16 with fp32 PSUM accumulation (measured end-to-end
    rel_l2 error vs fp32 ~4e-3).

Per-core device program, all on one NeuronCore:
  Phase 1: pre[bt, h] = x @ W_ih^T + (b_ih + b_hh), bt = t*32+b, stored bf16.
  Phase 2: 128 sequential steps; h kept transposed+packed as
           hT[p, kh*32+b] = h[b, kh*128+p] in one [128, 256] slab per step.
           Each psum column group ho gets pre injected via an
           identity-matmul (lhsT = pre slice, rhs = I32), then 8
           accumulating W_hh^T matmuls; one Tanh activation per step
           writes the packed slab straight into the H history buffer.
  Phase 3: out[v, bt] = W_out_shard @ H + b_out, 128-row vocab blocks,
           8 psum banks streaming 512-wide bt chunks.
"""

import numpy as np
import ml_dtypes

import concourse.bacc as bacc
import concourse.tile as tile
from concourse import mybir
from concourse.bass_utils import run_bass_kernel_spmd

B, T = 32, 128
E, H, V = 512, 1024, 32000
SOS_IDX = 1
NCORES = 8
VP = 32768            # padded vocab
VS = VP // NCORES     # vocab rows per core = 4096
BT = B * T            # 4096
BF16 = mybir.dt.bfloat16
F32 = mybir.dt.float32
_bf = ml_dtypes.bfloat16

_CACHE = {}


def _build():
    nc = bacc.Bacc(None, target_bir_lowering=False, debug=False)

    xT_d = nc.dram_tensor("xt", [E, BT], BF16, kind="ExternalInput")
    wih_d = nc.dram_tensor("wih", [E, H], BF16, kind="ExternalInput")
    whh_d = nc.dram_tensor("whh", [H, H], BF16, kind="ExternalInput")
    bb_d = nc.dram_tensor("bb", [128, H], BF16, kind="ExternalInput")
    wout_d = nc.dram_tensor("wout", [32, 128, 1024], BF16, kind="ExternalInput")
    bout_d = nc.dram_tensor("bout", [128, 32], F32, kind="ExternalInput")
    h0t_d = nc.dram_tensor("h0t", [128, 256], BF16, kind="ExternalInput")
    idn_d = nc.dram_tensor("idn", [128, 128], BF16, kind="ExternalInput")
    out_d = nc.dram_tensor("out", [VS, BT], F32, kind="ExternalOutput")

    ADD = mybir.AluOpType.add
    TANH = mybir.ActivationFunctionType.Tanh

    with tile.TileContext(nc) as tc:
        with tc.tile_pool(name="persist", bufs=1) as persist:
            Hs = persist.tile([128, T * 256], BF16)      # packed h history
            whh = persist.tile([128, 8 * H], BF16)       # 8 k-tiles of W_hh^T
            idn = persist.tile([128, 128], BF16)
            h0t = persist.tile([128, 256], BF16)
            bout = persist.tile([128, 32], F32)
            for kh in range(8):
                nc.sync.dma_start(whh[:, kh * H:(kh + 1) * H],
                                  whh_d[kh * 128:(kh + 1) * 128, :])
            nc.sync.dma_start(idn[:], idn_d[:])
            nc.sync.dma_start(h0t[:], h0t_d[:])
            nc.sync.dma_start(bout[:], bout_d[:])

            with tc.tile_pool(name="prep", bufs=1) as prep:
                pre = prep.tile([128, 32 * H], BF16)     # pre[btb | r, h]

                # ---- Phase 1: pre = x @ W_ih^T + b ----
                with (
                    tc.tile_pool(name="ph1", bufs=1) as ph1,
                    tc.tile_pool(name="xp", bufs=3) as xp,
                    tc.tile_pool(name="ps1", bufs=2, space="PSUM") as ps1,
                ):
                    wih = ph1.tile([128, 4 * H], BF16)   # 4 e-tiles of W_ih^T
                    bb = ph1.tile([128, H], BF16)
                    for e in range(4):
                        nc.sync.dma_start(wih[:, e * H:(e + 1) * H],
                                          wih_d[e * 128:(e + 1) * 128, :])
                    nc.sync.dma_start(bb[:], bb_d[:])
                    for btb in range(32):
                        xc = xp.tile([128, 512], BF16)
                        for e in range(4):
                            nc.sync.dma_start(
                                xc[:, e * 128:(e + 1) * 128],
                                xT_d[e * 128:(e + 1) * 128,
                                     btb * 128:(btb + 1) * 128])
                        for hc in range(2):
                            acc = ps1.tile([128, 512], F32)
                            for e in range(4):
                                nc.tensor.matmul(
                                    acc[:],
                                    xc[:, e * 128:(e + 1) * 128],
                                    wih[:, e * H + hc * 512: e * H + hc * 512 + 512],
                                    start=(e == 0), stop=(e == 3))
                            nc.vector.tensor_tensor(
                                pre[:, btb * H + hc * 512: btb * H + hc * 512 + 512],
                                acc[:], bb[:, hc * 512:(hc + 1) * 512], op=ADD)

                # ---- Phase 2: recurrence ----
                with tc.tile_pool(name="ps2", bufs=2, space="PSUM") as ps2:
                    for t in range(T):
                        ps = ps2.tile([128, 256], F32)
                        hprev = h0t[:] if t == 0 else Hs[:, (t - 1) * 256: t * 256]
                        pr = (t % 4) * 32
                        pcol = (t // 4) * H
                        for ho in range(8):
                            seg = ps[:, ho * 32:(ho + 1) * 32]
                            nc.tensor.matmul(
                                seg,
                                pre[:, pcol + ho * 128: pcol + ho * 128 + 128],
                                idn[:, pr:pr + 32],
                                start=True, stop=False)
                            for kh in range(8):
                                nc.tensor.matmul(
                                    seg,
                                    whh[:, kh * H + ho * 128: kh * H + ho * 128 + 128],
                                    hprev[:, kh * 32:(kh + 1) * 32],
                                    start=False, stop=(kh == 7))
                        nc.scalar.activation(Hs[:, t * 256:(t + 1) * 256], ps[:], TANH)

            # ---- Phase 3: out = W_out_shard @ H + b_out ----
            with (
                tc.tile_pool(name="wp", bufs=3) as wp,
                tc.tile_pool(name="st", bufs=8) as st,
                tc.tile_pool(name="ps3", bufs=4, space="PSUM") as ps3,
            ):
                Hr = Hs[:].rearrange("p (t c) -> p t c", c=256)
                for vb in range(32):
                    wt = wp.tile([128, 1024], BF16)
                    nc.sync.dma_start(wt[:], wout_d[vb])
                    for btc in range(8):
                        acc = ps3.tile([128, 512], F32)
                        for kh in range(8):
                            nc.tensor.matmul(
                                acc[:],
                                wt[:, kh * 128:(kh + 1) * 128],
                                Hr[:, btc * 16:(btc + 1) * 16, kh * 32:(kh + 1) * 32],
                                start=(kh == 0), stop=(kh == 7))
                        sg = st.tile([128, 512], F32)
                        nc.vector.tensor_scalar(
                            sg[:], acc[:], bout[:, vb:vb + 1], None, op0=ADD)
                        nc.sync.dma_start(
                            out_d[vb * 128:(vb + 1) * 128, btc * 512:(btc + 1) * 512],
                            sg[:])
    nc.compile()
    return nc


def _get_nc():
    if "nc" not in _CACHE:
        _CACHE["nc"] = _build()
    return _CACHE["nc"]


def _prep_inputs(target, h0, emb, W_ih, b_ih, W_hh, b_hh, W_out, b_out):
    target = np.asarray(target)
    h0 = np.asarray(h0, dtype=np.float32)
    emb = np.asarray(emb, dtype=np.float32)
    W_ih = np.asarray(W_ih, dtype=np.float32)
    b_ih = np.asarray(b_ih, dtype=np.float32)
    W_hh = np.asarray(W_hh, dtype=np.float32)
    b_hh = np.asarray(b_hh, dtype=np.float32)
    W_out = np.asarray(W_out, dtype=np.float32)
    b_out = np.asarray(b_out, dtype=np.float32)

    tokens = np.concatenate(
        [np.full((B, 1), SOS_IDX, dtype=target.dtype), target[:, 1:]], axis=1)
    x = emb[tokens]                                   # (B, T, E) f32
    # xT[e, t*B + b] = x[b, t, e]
    xT = np.ascontiguousarray(x.transpose(2, 1, 0).reshape(E, BT)).astype(_bf)
    wihT = np.ascontiguousarray(W_ih.T).astype(_bf)   # (E, H)
    whhT = np.ascontiguousarray(W_hh.T).astype(_bf)   # (H, H)
    bb = np.broadcast_to((b_ih + b_hh).astype(_bf), (128, H))
    bb = np.ascontiguousarray(bb)
    # h0t[p, kh*32 + b] = h0[b, kh*128 + p]
    h0t = np.ascontiguousarray(
        h0.reshape(B, 8, 128).transpose(2, 1, 0).reshape(128, 256)).astype(_bf)
    idn = np.eye(128, dtype=_bf)

    Wp = np.zeros((VP, H), dtype=np.float32)
    Wp[:V] = W_out
    bp = np.zeros((VP,), dtype=np.float32)
    bp[:V] = b_out

    shared = dict(xt=xT, wih=wihT, whh=whhT, bb=bb, h0t=h0t, idn=idn)
    in_maps = []
    for c in range(NCORES):
        ws = Wp[c * VS:(c + 1) * VS]                  # (4096, 1024)
        # wout[vb, p, kh*128 + m] = ws[vb*128 + m, kh*128 + p]
        wr = np.ascontiguousarray(
            ws.reshape(32, 128, 8, 128).transpose(0, 3, 2, 1).reshape(32, 128, 1024)
        ).astype(_bf)
        bs = np.ascontiguousarray(
            bp[c * VS:(c + 1) * VS].reshape(32, 128).T)  # (128, 32)
        in_maps.append(dict(shared, wout=wr, bout=bs))
    return in_maps


def kernel(target, h0, emb, W_ih, b_ih, W_hh, b_hh, W_out, b_out):
    nc = _get_nc()
    in_maps = _prep_inputs(target, h0, emb, W_ih, b_ih, W_hh, b_hh, W_out, b_out)
    _CACHE["last_in_maps"] = in_maps
    res = run_bass_kernel_spmd(nc, in_maps, core_ids=list(range(NCORES)))
    _CACHE["last_result"] = res
    shards = [res.results[c]["out"] for c in range(NCORES)]
    full = np.concatenate(shards, axis=0)[:V]         # (V, BT), bt = t*B + b
    out = full.reshape(V, T, B).transpose(2, 1, 0)    # (B, T, V)
    return np.ascontiguousarray(out)
